# revision 5
# baseline (speedup 1.0000x reference)
"""RWKV-style WKV attention kernel for 8 TRN2 NeuronCores.

Strategy (channel-parallel, zero collectives):
  - The D=256 channel dim is sharded 32-per-core. Each core computes its
    32 channels of k/v/r via one PSUM-accumulated matmul chain (time-mix is
    folded into pre-scaled stacked weights; the time-shift xs is a shifted
    window into the same x^T buffer), runs the WKV recurrence with the DVE
    tensor_tensor_scan instruction, and emits a partial output projection
    (x @ Wo[:, shard].T). The 8 partial outputs are summed on the host.
"""

import numpy as np

import concourse.bass as bass
import concourse.mybir as mybir
import concourse.tile as tile
from concourse import bacc
from concourse.bass_utils import run_bass_kernel_spmd

B, T, D = 2, 512, 256
NCORES = 8
DLOC = D // NCORES  # 32 channels per core
F32 = mybir.dt.float32


def build_nc():
    nc = bacc.Bacc(None, target_bir_lowering=False)

    xT = nc.declare_dram_parameter("xT", [D, B * (T + 1)], F32, isOutput=False)
    wkvr = nc.declare_dram_parameter("wkvr", [2 * D, 3 * DLOC], F32, isOutput=False)
    woT = nc.declare_dram_parameter("woT", [DLOC, D], F32, isOutput=False)
    ab = nc.declare_dram_parameter("abcast", [DLOC, T], F32, isOutput=False)
    etf = nc.declare_dram_parameter("etf", [DLOC, 1], F32, isOutput=False)
    lnum = nc.declare_dram_parameter("lastnum", [DLOC, B], F32, isOutput=False)
    lden = nc.declare_dram_parameter("lastden", [DLOC, B], F32, isOutput=False)
    outp = nc.declare_dram_parameter("out_part", [B, T, D], F32, isOutput=True)
    ndl = nc.declare_dram_parameter("nd_last", [DLOC, 2 * B], F32, isOutput=True)

    Exp = mybir.ActivationFunctionType.Exp
    Sig = mybir.ActivationFunctionType.Sigmoid
    mult = mybir.AluOpType.mult
    add = mybir.AluOpType.add

    with tile.TileContext(nc) as tc:
        with (
            tc.tile_pool(name="sb", bufs=1) as sb,
            tc.tile_pool(name="psk", bufs=2, space="PSUM") as psk,
            tc.tile_pool(name="pso", bufs=4, space="PSUM") as pso,
        ):
            # ---- input DMAs ----
            xt = []
            for dt in range(2):
                t_ = sb.tile([128, B * (T + 1)], F32, tag=f"xt{dt}")
                nc.sync.dma_start(out=t_, in_=xT[dt * 128 : (dt + 1) * 128, :])
                xt.append(t_)
            wt = []
            for kt in range(4):
                t_ = sb.tile([128, 3 * DLOC], F32, tag=f"wt{kt}")
                nc.sync.dma_start(out=t_, in_=wkvr[kt * 128 : (kt + 1) * 128, :])
                wt.append(t_)
            wo_t = sb.tile([DLOC, D], F32)
            nc.sync.dma_start(out=wo_t, in_=woT[:, :])
            ab_t = sb.tile([DLOC, T], F32)
            nc.sync.dma_start(out=ab_t, in_=ab[:, :])
            etf_t = sb.tile([DLOC, 1], F32)
            nc.sync.dma_start(out=etf_t, in_=etf[:, :])
            ln_t = sb.tile([DLOC, B], F32)
            nc.sync.dma_start(out=ln_t, in_=lnum[:, :])
            ld_t = sb.tile([DLOC, B], F32)
            nc.sync.dma_start(out=ld_t, in_=lden[:, :])

            # ---- working tiles ----
            ek = sb.tile([DLOC, B, T], F32)
            srb = sb.tile([DLOC, B, T], F32)
            unum = sb.tile([DLOC, B, T], F32)
            numb = sb.tile([DLOC, B, T + 1], F32)
            denb = sb.tile([DLOC, B, T + 1], F32)
            nmr = sb.tile([DLOC, B, T], F32)
            dnm = sb.tile([DLOC, B, T], F32)
            rcp = sb.tile([DLOC, B, T], F32)
            wsr = sb.tile([DLOC, B, T], F32)

            for b in range(B):
                ps = psk.tile([3 * DLOC, T], F32, tag="kvr")
                base = b * (T + 1)
                for kt in range(4):
                    dtile = kt % 2
                    shift = 1 if kt < 2 else 0  # x view vs time-shifted xs view
                    rhs = xt[dtile][:, base + shift : base + shift + T]
                    nc.tensor.matmul(ps, wt[kt], rhs, start=(kt == 0), stop=(kt == 3))

                nc.scalar.activation(ek[:, b, :], ps[0:DLOC, :], Exp)
                nc.scalar.activation(srb[:, b, :], ps[64:96, :], Sig)
                nc.vector.tensor_mul(unum[:, b, :], ek[:, b, :], ps[DLOC : 2 * DLOC, :])
                nc.vector.tensor_copy(numb[:, b, 0:1], ln_t[:, b : b + 1])
                nc.vector.tensor_copy(denb[:, b, 0:1], ld_t[:, b : b + 1])
                nc.vector.tensor_tensor_scan(
                    numb[:, b, 1 : T + 1], ab_t, unum[:, b, :],
                    ln_t[:, b : b + 1], mult, add,
                )
                nc.vector.tensor_tensor_scan(
                    denb[:, b, 1 : T + 1], ab_t, ek[:, b, :],
                    ld_t[:, b : b + 1], mult, add,
                )

            # merged epilogue over both batches
            nc.vector.scalar_tensor_tensor(
                nmr[:, :, :], unum[:, :, :], etf_t[:, 0:1], numb[:, :, 0:T], mult, add
            )
            nc.vector.scalar_tensor_tensor(
                dnm[:, :, :], ek[:, :, :], etf_t[:, 0:1], denb[:, :, 0:T], mult, add
            )
            nc.vector.reciprocal(rcp[:, :, :], dnm[:, :, :])
            nc.vector.tensor_mul(wsr[:, :, :], nmr[:, :, :], srb[:, :, :])
            nc.vector.tensor_mul(wsr[:, :, :], wsr[:, :, :], rcp[:, :, :])

            # partial output projection: out[t, :] += wkvsr[t, shard] @ Wo[:, shard].T
            for b in range(B):
                for tt in range(4):
                    po = pso.tile([128, D], F32, tag="po")
                    nc.tensor.matmul(
                        po, wsr[:, b, tt * 128 : (tt + 1) * 128], wo_t,
                        start=True, stop=True,
                    )
                    ob = sb.tile([128, D], F32, tag=f"ob{b}_{tt}")
                    nc.any.tensor_copy(ob, po)
                    nc.sync.dma_start(
                        out=outp[b, tt * 128 : (tt + 1) * 128, :], in_=ob
                    )

            nc.sync.dma_start(out=ndl[:, 0:B], in_=numb[:, :, T])
            nc.sync.dma_start(out=ndl[:, B : 2 * B], in_=denb[:, :, T])

    nc.compile()
    return nc


_NC_CACHE = None


def _get_nc():
    global _NC_CACHE
    if _NC_CACHE is None:
        _NC_CACHE = build_nc()
    return _NC_CACHE


def prepare_in_maps(inputs):
    x = np.asarray(inputs["x"], np.float32)
    last_x = np.asarray(inputs["last_x"], np.float32)
    last_num = np.asarray(inputs["last_num"], np.float32)
    last_den = np.asarray(inputs["last_den"], np.float32)
    td = np.asarray(inputs["time_decay"], np.float32)
    tf = np.asarray(inputs["time_first"], np.float32)
    mk = np.asarray(inputs["time_mix_k"], np.float32).reshape(D)
    mv = np.asarray(inputs["time_mix_v"], np.float32).reshape(D)
    mr = np.asarray(inputs["time_mix_r"], np.float32).reshape(D)
    Wk = np.asarray(inputs["Wk"], np.float32)
    Wv = np.asarray(inputs["Wv"], np.float32)
    Wr = np.asarray(inputs["Wr"], np.float32)
    Wo = np.asarray(inputs["Wo"], np.float32)

    # x^T with the time-shift boundary column per batch:
    # col b*(T+1) = last_x[b], cols b*(T+1)+1.. = x[b].T
    xT = np.empty((D, B * (T + 1)), np.float32)
    for b in range(B):
        xT[:, b * (T + 1)] = last_x[b, 0, :]
        xT[:, b * (T + 1) + 1 : (b + 1) * (T + 1)] = x[b].T

    a = np.exp(-np.exp(td, dtype=np.float64)).astype(np.float32)
    etf_full = np.exp(tf).astype(np.float32)

    in_maps = []
    for c in range(NCORES):
        sh = slice(c * DLOC, (c + 1) * DLOC)
        wx = np.concatenate(
            [(Wk[sh, :] * mk[None, :]).T,
             (Wv[sh, :] * mv[None, :]).T,
             (Wr[sh, :] * mr[None, :]).T], axis=1)
        wxs = np.concatenate(
            [(Wk[sh, :] * (1.0 - mk)[None, :]).T,
             (Wv[sh, :] * (1.0 - mv)[None, :]).T,
             (Wr[sh, :] * (1.0 - mr)[None, :]).T], axis=1)
        in_maps.append({
            "xT": xT,
            "wkvr": np.ascontiguousarray(
                np.concatenate([wx, wxs], axis=0), dtype=np.float32),
            "woT": np.ascontiguousarray(Wo[:, sh].T, dtype=np.float32),
            "abcast": np.ascontiguousarray(
                np.repeat(a[sh][:, None], T, axis=1), dtype=np.float32),
            "etf": np.ascontiguousarray(etf_full[sh][:, None], dtype=np.float32),
            "lastnum": np.ascontiguousarray(last_num[:, 0, sh].T, dtype=np.float32),
            "lastden": np.ascontiguousarray(last_den[:, 0, sh].T, dtype=np.float32),
        })
    return in_maps


def postprocess(results, inputs):
    x = np.asarray(inputs["x"], np.float32)
    out = np.zeros((B, T, D), np.float32)
    num_l = np.empty((B, 1, D), np.float32)
    den_l = np.empty((B, 1, D), np.float32)
    for c in range(NCORES):
        out += results[c]["out_part"].reshape(B, T, D)
        nd = results[c]["nd_last"].reshape(DLOC, 2 * B)
        sh = slice(c * DLOC, (c + 1) * DLOC)
        num_l[:, 0, sh] = nd[:, 0:B].T
        den_l[:, 0, sh] = nd[:, B : 2 * B].T
    x_last = np.ascontiguousarray(x[:, -1:, :])
    return out, x_last, num_l, den_l


def kernel(**inputs):
    nc = _get_nc()
    in_maps = prepare_in_maps(inputs)
    res = run_bass_kernel_spmd(nc, in_maps, list(range(NCORES)))
    return postprocess(res.results, inputs)


# revision 12
# speedup vs baseline: 1.5600x; 1.5600x over previous
"""RWKV-style WKV attention kernel for 8 TRN2 NeuronCores.

Strategy (channel-parallel, zero collectives):
  - The D=256 channel dim is sharded 32-per-core. Each core computes its
    32 channels of k/v/r via one PSUM-accumulated bf16 matmul chain (time-mix
    folded into pre-scaled stacked weights; the time-shift xs is a shifted
    window into the same x^T buffer), runs the WKV recurrence with the DVE
    tensor_tensor_scan instruction (batches merged on partitions: rows
    b*32+ch), and emits a partial output projection (wkvsr @ Wo[:, shard].T)
    in bf16. The 8 partial outputs are summed on the host.
"""

import ml_dtypes
import numpy as np

import concourse.bass as bass
import concourse.mybir as mybir
import concourse.tile as tile
from concourse import bacc
from concourse.bass_utils import run_bass_kernel_spmd

B, T, D = 2, 512, 256
NCORES = 8
DLOC = D // NCORES  # 32 channels per core
P = B * DLOC  # 64 partition rows: (b, ch_local)
F32 = mybir.dt.float32
BF16 = mybir.dt.bfloat16


def build_nc():
    nc = bacc.Bacc(None, target_bir_lowering=False)

    xT = nc.declare_dram_parameter("xT", [D, B * (T + 1)], BF16, isOutput=False)
    wkvr = nc.declare_dram_parameter("wkvr", [2 * D, 3 * DLOC], BF16, isOutput=False)
    woT = nc.declare_dram_parameter("woT", [DLOC, D], BF16, isOutput=False)
    ab = nc.declare_dram_parameter("abcast", [P, T], F32, isOutput=False)
    etf = nc.declare_dram_parameter("etf", [P, 1], F32, isOutput=False)
    lnum = nc.declare_dram_parameter("lastnum", [P, 1], F32, isOutput=False)
    lden = nc.declare_dram_parameter("lastden", [P, 1], F32, isOutput=False)
    outp = nc.declare_dram_parameter("out_part", [B, T, D], BF16, isOutput=True)
    ndl = nc.declare_dram_parameter("nd_last", [P, 2], F32, isOutput=True)

    Exp = mybir.ActivationFunctionType.Exp
    Sig = mybir.ActivationFunctionType.Sigmoid
    mult = mybir.AluOpType.mult
    add = mybir.AluOpType.add

    with tile.TileContext(nc) as tc:
        with (
            tc.tile_pool(name="sb", bufs=1) as sb,
            tc.tile_pool(name="psk", bufs=2, space="PSUM") as psk,
            tc.tile_pool(name="pso", bufs=4, space="PSUM") as pso,
        ):
            # ---- input DMAs, spread across engine queues ----
            xt = []
            for dt, eng in ((0, nc.sync), (1, nc.gpsimd)):
                t_ = sb.tile([128, B * (T + 1)], BF16, tag=f"xt{dt}")
                eng.dma_start(out=t_, in_=xT[dt * 128 : (dt + 1) * 128, :])
                xt.append(t_)
            wt = []
            for kt in range(4):
                t_ = sb.tile([128, 3 * DLOC], BF16, tag=f"wt{kt}")
                nc.scalar.dma_start(out=t_, in_=wkvr[kt * 128 : (kt + 1) * 128, :])
                wt.append(t_)
            wo_t = sb.tile([DLOC, D], BF16)
            nc.scalar.dma_start(out=wo_t, in_=woT[:, :])
            ab_t = sb.tile([P, T], F32)
            nc.sync.dma_start(out=ab_t, in_=ab[:, :])
            etf_t = sb.tile([P, 1], F32)
            nc.scalar.dma_start(out=etf_t, in_=etf[:, :])

            # ---- working tiles ([P, T]: batches merged on partitions) ----
            ek = sb.tile([P, T], F32)
            srb = sb.tile([P, T], F32)
            un = sb.tile([P, T], F32)
            numb = sb.tile([P, T + 1], F32)
            denb = sb.tile([P, T + 1], F32)
            nmr = sb.tile([P, T], F32)
            dnm = sb.tile([P, T], F32)
            rcp = sb.tile([P, T], F32)
            tns = sb.tile([P, T], F32)
            wsrb0 = sb.tile([DLOC, T], BF16)
            wsrb1 = sb.tile([DLOC, T], BF16)
            wsrb = [wsrb0, wsrb1]

            # scan seeds: DMA the incoming state straight into column 0
            nc.gpsimd.dma_start(out=numb[:, 0:1], in_=lnum[:, :])
            nc.scalar.dma_start(out=denb[:, 0:1], in_=lden[:, :])

            # warm the Exp table while DMAs run
            dummy = sb.tile([1, 1], F32)
            nc.vector.memset(dummy, 0.0)
            nc.scalar.activation(dummy, dummy, Exp)

            # ---- k/v/r projection matmuls (bf16 in, f32 accumulate) ----
            pss = []
            for b in range(B):
                ps = psk.tile([3 * DLOC, T], F32, tag="kvr")
                base = b * (T + 1)
                for kt in range(4):
                    dtile = kt % 2
                    shift = 1 if kt < 2 else 0  # x view vs time-shifted xs view
                    rhs = xt[dtile][:, base + shift : base + shift + T]
                    nc.tensor.matmul(ps, wt[kt], rhs, start=(kt == 0), stop=(kt == 3))
                pss.append(ps)

            # ---- activations, batched by function to avoid table reloads ----
            for b in range(B):
                nc.scalar.activation(ek[b * DLOC : (b + 1) * DLOC, :],
                                     pss[b][0:DLOC, :], Exp)
            for b in range(B):
                nc.scalar.activation(srb[b * DLOC : (b + 1) * DLOC, :],
                                     pss[b][64:96, :], Sig)

            # ---- WKV recurrence ----
            for b in range(B):
                nc.vector.tensor_mul(un[b * DLOC : (b + 1) * DLOC, :],
                                     ek[b * DLOC : (b + 1) * DLOC, :],
                                     pss[b][DLOC : 2 * DLOC, :])
            nc.vector.tensor_tensor_scan(
                numb[:, 1 : T + 1], ab_t, un, numb[:, 0:1], mult, add)
            nc.vector.tensor_tensor_scan(
                denb[:, 1 : T + 1], ab_t, ek, denb[:, 0:1], mult, add)

            nc.vector.scalar_tensor_tensor(
                dnm, ek, etf_t[:, 0:1], denb[:, 0:T], mult, add)
            nc.vector.scalar_tensor_tensor(
                nmr, un, etf_t[:, 0:1], numb[:, 0:T], mult, add)
            nc.vector.reciprocal_approx_fast(rcp, dnm)
            nc.gpsimd.tensor_mul(tns, nmr, srb)
            for b in range(B):
                nc.vector.tensor_mul(wsrb[b],
                                     tns[b * DLOC : (b + 1) * DLOC, :],
                                     rcp[b * DLOC : (b + 1) * DLOC, :])

            # ---- partial output projection (bf16) ----
            out_dma = [nc.sync, nc.gpsimd, nc.scalar]
            for b in range(B):
                for tt in range(4):
                    po = pso.tile([128, D], F32, tag="po")
                    nc.tensor.matmul(
                        po, wsrb[b][:, tt * 128 : (tt + 1) * 128],
                        wo_t, start=True, stop=True,
                    )
                    ob = sb.tile([128, D], BF16, tag=f"ob{b}_{tt}")
                    nc.any.tensor_copy(ob, po)
                    out_dma[(b * 4 + tt) % 3].dma_start(
                        out=outp[b, tt * 128 : (tt + 1) * 128, :], in_=ob
                    )

            nc.sync.dma_start(out=ndl[:, 0:1], in_=numb[:, T : T + 1])
            nc.scalar.dma_start(out=ndl[:, 1:2], in_=denb[:, T : T + 1])

    nc.compile()
    return nc


_NC_CACHE = None


def _get_nc():
    global _NC_CACHE
    if _NC_CACHE is None:
        _NC_CACHE = build_nc()
    return _NC_CACHE


def prepare_in_maps(inputs):
    x = np.asarray(inputs["x"], np.float32)
    last_x = np.asarray(inputs["last_x"], np.float32)
    last_num = np.asarray(inputs["last_num"], np.float32)
    last_den = np.asarray(inputs["last_den"], np.float32)
    td = np.asarray(inputs["time_decay"], np.float32)
    tf = np.asarray(inputs["time_first"], np.float32)
    mk = np.asarray(inputs["time_mix_k"], np.float32).reshape(D)
    mv = np.asarray(inputs["time_mix_v"], np.float32).reshape(D)
    mr = np.asarray(inputs["time_mix_r"], np.float32).reshape(D)
    Wk = np.asarray(inputs["Wk"], np.float32)
    Wv = np.asarray(inputs["Wv"], np.float32)
    Wr = np.asarray(inputs["Wr"], np.float32)
    Wo = np.asarray(inputs["Wo"], np.float32)

    # x^T with the time-shift boundary column per batch:
    # col b*(T+1) = last_x[b], cols b*(T+1)+1.. = x[b].T
    xT = np.empty((D, B * (T + 1)), np.float32)
    for b in range(B):
        xT[:, b * (T + 1)] = last_x[b, 0, :]
        xT[:, b * (T + 1) + 1 : (b + 1) * (T + 1)] = x[b].T
    xT = xT.astype(ml_dtypes.bfloat16)

    a = np.exp(-np.exp(td, dtype=np.float64)).astype(np.float32)
    etf_full = np.exp(tf).astype(np.float32)

    in_maps = []
    for c in range(NCORES):
        sh = slice(c * DLOC, (c + 1) * DLOC)
        wx = np.concatenate(
            [(Wk[sh, :] * mk[None, :]).T,
             (Wv[sh, :] * mv[None, :]).T,
             (Wr[sh, :] * mr[None, :]).T], axis=1)
        wxs = np.concatenate(
            [(Wk[sh, :] * (1.0 - mk)[None, :]).T,
             (Wv[sh, :] * (1.0 - mv)[None, :]).T,
             (Wr[sh, :] * (1.0 - mr)[None, :]).T], axis=1)
        # [P]-layout params: row b*DLOC+j  <->  channel sh[j] of batch b
        a_p = np.tile(a[sh], B)
        etf_p = np.tile(etf_full[sh], B)
        ln_p = last_num[:, 0, sh].reshape(P)
        ld_p = last_den[:, 0, sh].reshape(P)
        in_maps.append({
            "xT": xT,
            "wkvr": np.ascontiguousarray(
                np.concatenate([wx, wxs], axis=0)).astype(ml_dtypes.bfloat16),
            "woT": np.ascontiguousarray(Wo[:, sh].T).astype(ml_dtypes.bfloat16),
            "abcast": np.ascontiguousarray(
                np.repeat(a_p[:, None], T, axis=1), dtype=np.float32),
            "etf": np.ascontiguousarray(etf_p[:, None], dtype=np.float32),
            "lastnum": np.ascontiguousarray(ln_p[:, None], dtype=np.float32),
            "lastden": np.ascontiguousarray(ld_p[:, None], dtype=np.float32),
        })
    return in_maps


def postprocess(results, inputs):
    x = np.asarray(inputs["x"], np.float32)
    out = np.zeros((B, T, D), np.float32)
    num_l = np.empty((B, 1, D), np.float32)
    den_l = np.empty((B, 1, D), np.float32)
    for c in range(NCORES):
        out += results[c]["out_part"].reshape(B, T, D).astype(np.float32)
        nd = np.asarray(results[c]["nd_last"], np.float32).reshape(P, 2)
        sh = slice(c * DLOC, (c + 1) * DLOC)
        num_l[:, 0, sh] = nd[:, 0].reshape(B, DLOC)
        den_l[:, 0, sh] = nd[:, 1].reshape(B, DLOC)
    x_last = np.ascontiguousarray(x[:, -1:, :])
    return out, x_last, num_l, den_l


def kernel(**inputs):
    nc = _get_nc()
    in_maps = prepare_in_maps(inputs)
    res = run_bass_kernel_spmd(nc, in_maps, list(range(NCORES)))
    return postprocess(res.results, inputs)
